# revision 44
# baseline (speedup 1.0000x reference)
"""Chunked DeltaNet layer on 8 TRN2 NeuronCores.

Sharding: core c -> batch b = c//2, head-group hh = c%2 (heads hh*4..hh*4+3).
Each core: q/k/v projections + causal conv + normalization for its 4 heads,
chunked (WY-form) delta rule over L=2048 in 16 chunks of 128, RMS norm,
partial out-projection [2048, 1024] (contraction over its 512 local o-dims).
Host sums the two partials per batch and adds bo.

Chunk math (per head, state S = A^T in [d_k, d_v]):
  N    = tril(K' K^T, -1)          K' = beta'*K_n (row scale), K_n = K/||K||
  Minv ~= (I+N^4)(I+N^2)(I-N)      [error ~N^8, ~5e-4 on this data]
  U    = Minv (V' - K' S)          (R-form: chain runs on one 128-wide tile)
  O    = diag(r_q) [tril(Q_raw K_n^T, 0) U + Q_raw S]
  S   += K_n^T U

Engine split: PE matmuls/transposes; DVE normalization scales, Neumann adds,
row-sum reductions (tensor_tensor_reduce); ACT psum drains + sigmoid/sqrt;
GpSimd bulk psum->SBUF copies (vkt/s16/ot).

Emission is software-pipelined per chunk c:
  proj-piece(lt+1) | A1(c+1) transposes+grams | SC(c+1) scalars |
  A2(c) Neumann chain | B1(c) outputs+state | outproj(c-1) | B2(c) RMS drain
so the PE FIFO always holds ready work while ACT/DVE chains resolve.
"""

import contextlib
import os

import ml_dtypes
import numpy as np

import concourse.bass as bass
import concourse.mybir as mybir
import concourse.tile as tile
from concourse import bacc

F32 = mybir.dt.float32
BF16 = mybir.dt.bfloat16
AF = mybir.ActivationFunctionType
ALU = mybir.AluOpType
AX = mybir.AxisListType

B, L, D, H, HD, CONV = 4, 2048, 1024, 8, 128, 4
ETA, EPS = 1.0, 1e-6
C = 128
NCH = L // C
NLT = 4
LT = 512
HL = 4
KS = D // 128
SIG = ("k", "q", "v")


def build_nc():
    nc = bacc.Bacc("TRN2", target_bir_lowering=False, debug=False)

    xt_d = nc.dram_tensor("xt", [KS, 128, L], BF16, kind="ExternalInput").ap()
    wq_d = nc.dram_tensor("wq", [KS, 128, 512], BF16, kind="ExternalInput").ap()
    wk_d = nc.dram_tensor("wk", [KS, 128, 512], BF16, kind="ExternalInput").ap()
    wv_d = nc.dram_tensor("wv", [KS, 128, 512], BF16, kind="ExternalInput").ap()
    wb_d = nc.dram_tensor("wb", [KS, 128, 4], BF16, kind="ExternalInput").ap()
    wo_d = nc.dram_tensor("wo", [4, 128, 1024], BF16, kind="ExternalInput").ap()
    cd_d = nc.dram_tensor("cd", [12, CONV, 128, 128], BF16, kind="ExternalInput").ap()
    mk_d = nc.dram_tensor("mk", [3, 128, 128], F32, kind="ExternalInput").ap()
    oh_d = nc.dram_tensor("oh", [4, 128, 4], BF16, kind="ExternalInput").ap()
    id16_d = nc.dram_tensor("id16", [128, 128], BF16, kind="ExternalInput").ap()
    id32_d = nc.dram_tensor("id32", [128, 128], F32, kind="ExternalInput").ap()
    out_d = nc.dram_tensor("out", [L, D], F32, kind="ExternalOutput").ap()

    with tile.TileContext(nc) as tc, contextlib.ExitStack() as ctx:
        consts = ctx.enter_context(tc.tile_pool(name="consts", bufs=1))
        persist = ctx.enter_context(tc.tile_pool(name="persist", bufs=1))
        projp = ctx.enter_context(tc.tile_pool(name="projp", bufs=2))
        rawp = ctx.enter_context(tc.tile_pool(name="rawp", bufs=2))
        sqp = ctx.enter_context(tc.tile_pool(name="sqp", bufs=2))
        workp = ctx.enter_context(tc.tile_pool(name="workp", bufs=2))
        outp = ctx.enter_context(tc.tile_pool(name="outp", bufs=2))
        # PSUM: 8 banks of 2KB/partition, each pool buf = 1 bank.
        # big 2 + pst 1 + kq 1 + mm 2 + po 1 + ptx 1 = 8 banks.
        # Per-head [128,128] outputs pack into [128,4,128] bank tiles.
        ps_big = ctx.enter_context(tc.tile_pool(name="ps_big", bufs=2, space="PSUM"))
        ps_pst = ctx.enter_context(tc.tile_pool(name="ps_pst", bufs=1, space="PSUM"))
        ps_op = ctx.enter_context(tc.tile_pool(name="ps_op", bufs=1, space="PSUM"))
        ps_mm = ctx.enter_context(tc.tile_pool(name="ps_mm", bufs=2, space="PSUM"))
        ps_po = ctx.enter_context(tc.tile_pool(name="ps_po", bufs=1, space="PSUM"))
        ps_ptx = ctx.enter_context(tc.tile_pool(name="ps_ptx", bufs=1, space="PSUM"))

        # ---- constants, DMA-ordered by first use ----
        xt = consts.tile([128, KS, L], BF16)
        ws = {}
        for name in ("k", "q", "v"):
            ws[name] = consts.tile([128, KS, 512], BF16, name=f"w{name}")
        cd = consts.tile([128, 12, CONV, 128], BF16)
        mk = consts.tile([128, 3, 128], F32)
        oh = consts.tile([128, 4, 4], BF16)
        id16 = consts.tile([128, 128], BF16)
        id32 = consts.tile([128, 128], F32)
        wb = consts.tile([128, KS, 4], BF16)
        wo = consts.tile([128, 4, 1024], BF16)

        wd = {"k": wk_d, "q": wq_d, "v": wv_d}
        for i in range(KS):
            for sp in range(2):
                ss = bass.ds(sp * (L // 2), L // 2)
                nc.sync.dma_start(out=xt[:, i, ss], in_=xt_d[i][:, ss])
            nc.sync.dma_start(out=ws["k"][:, i, :], in_=wk_d[i])
        for n_ in range(4):
            for j_ in range(CONV):
                nc.sync.dma_start(out=cd[:, n_, j_, :], in_=cd_d[n_, j_])
        nc.sync.dma_start(out=id16, in_=id16_d)
        nc.sync.dma_start(out=id32, in_=id32_d)
        for n_ in range(3):
            nc.sync.dma_start(out=mk[:, n_, :], in_=mk_d[n_])
        for i in range(KS):
            nc.sync.dma_start(out=ws["q"][:, i, :], in_=wq_d[i])
        for n_ in range(4, 8):
            for j_ in range(CONV):
                nc.sync.dma_start(out=cd[:, n_, j_, :], in_=cd_d[n_, j_])
        for i in range(KS):
            nc.sync.dma_start(out=ws["v"][:, i, :], in_=wv_d[i])
        for n_ in range(8, 12):
            for j_ in range(CONV):
                nc.sync.dma_start(out=cd[:, n_, j_, :], in_=cd_d[n_, j_])
        for n_ in range(4):
            nc.sync.dma_start(out=oh[:, n_, :], in_=oh_d[n_])
        for i in range(KS):
            nc.sync.dma_start(out=wb[:, i, :], in_=wb_d[i])
        for i in range(4):
            nc.sync.dma_start(out=wo[:, i, :], in_=wo_d[i])

        # ---- persistent ----
        beta_cm = persist.tile([4, L], F32)  # sigmoid(beta)
        qn2_cm = persist.tile([4, L], F32)   # ||q_raw||^2
        s32 = persist.tile([128, HL, 128], F32)
        s16 = persist.tile([128, HL, 128], BF16)
        halo = persist.tile([128, 12, 4], BF16)  # conv halo between lt blocks
        nc.vector.memset(s32, 0.0)
        nc.vector.memset(s16, 0.0)
        epsb = persist.tile([128, 1], F32)
        nc.vector.memset(epsb, EPS)

        raws = [{} for _ in range(NLT)]
        sqs = [{} for _ in range(NLT)]
        st = [None] * NCH  # per-chunk state dicts

        # one sig per piece so ACT Sigmoid ops cluster (fewer table loads)
        PIECES = [[("k", h) for h in range(HL)], [("q", h) for h in range(HL)],
                  [("v", h) for h in range(HL)], []]

        def emit_proj_piece(lt, piece):
            """Projection+conv for one sig (4 heads); piece 2 adds beta/qn2."""
            tsl = bass.ds(lt * LT, LT)
            raw = raws[lt]
            items = PIECES[piece]
            pts = []
            for s, h in items:
                n = SIG.index(s) * HL + h
                ps = ps_big.tile([128, LT], F32, name="psproj", tag="big")
                for i in range(KS):
                    nc.tensor.matmul(
                        ps, ws[s][:, i, h * 128:(h + 1) * 128], xt[:, i, tsl],
                        start=(i == 0), stop=(i == KS - 1))
                pt = projp.tile([128, LT + 4], BF16, name="pt", tag=f"pj{s}{h}",
                                bufs=1)
                if lt == 0:
                    nc.scalar.memzero(pt[:, 0:4])
                else:
                    nc.scalar.copy(pt[:, 0:3], halo[:, n, 0:3])
                nc.scalar.copy(pt[:, 3:LT + 3], ps)
                if lt + 1 < NLT:
                    nc.scalar.copy(halo[:, n, 0:3], pt[:, LT:LT + 3])
                pts.append(pt)
            for (s, h), pt in zip(items, pts):
                n = SIG.index(s) * HL + h
                pc = ps_big.tile([128, LT], F32, name="psconv", tag="big")
                for j in range(CONV):
                    nc.tensor.matmul(pc, cd[:, n, j, :], pt[:, j:LT + j],
                                     start=(j == 0), stop=(j == CONV - 1))
                r = rawp.tile([128, LT], BF16, name="raw", tag=f"rw{s}{h}")
                if s == "v":
                    vsg = sqp.tile([128, LT], BF16, name="vsig", tag=f"vg{h}",
                                   bufs=1)
                    nc.scalar.activation(vsg, pc, AF.Sigmoid)
                    nc.vector.tensor_mul(r, pc, vsg)  # silu(v)
                else:
                    nc.scalar.copy(r, pc)
                    if s == "q":
                        sq = sqp.tile([128, LT], BF16, name="sq", tag=f"sq{h}",
                                      bufs=1)
                        nc.vector.tensor_mul(sq, r, r)
                        sqs[lt][h] = sq
                raw[(s, h)] = r
            if piece == 2:
                psq = ps_big.tile([128, LT], F32, name="psq", tag="big")
                for h in range(HL):
                    nc.tensor.matmul(psq[0:4, :], oh[:, h, :], sqs[lt][h],
                                     start=(h == 0), stop=(h == HL - 1))
                nc.vector.tensor_copy(qn2_cm[:, tsl], psq[0:4, :])
                psb = ps_big.tile([128, LT], F32, name="psbeta", tag="big")
                for i in range(KS):
                    nc.tensor.matmul(psb[0:4, :], wb[:, i, :], xt[:, i, tsl],
                                     start=(i == 0), stop=(i == KS - 1))
                nc.scalar.activation(beta_cm[:, tsl], psb[0:4, :], AF.Sigmoid)

        def emit_A1(cidx):
            """Transposes + norm stats for chunk cidx (independent of scalars)."""
            lt = cidx // 4
            raw = raws[lt]
            csl = bass.ds((cidx % 4) * C, C)
            gsl = bass.ds(cidx * C, C)

            psbt = ps_mm.tile([128, 8], F32, name="psbt", tag="mm")
            nc.tensor.transpose(psbt[:, 0:4], beta_cm[:, gsl], id32[0:4, 0:4])
            nc.tensor.transpose(psbt[:, 4:8], qn2_cm[:, gsl], id32[0:4, 0:4])

            nq = workp.tile([128, 8], F32, name="nq", tag="nq")
            nc.vector.tensor_copy(nq[:, 4:8], psbt[:, 4:8])
            bt = workp.tile([128, 4], F32, name="bt", tag="bt")
            nc.vector.tensor_copy(bt, psbt[:, 0:4])

            pkv = ps_pst.tile([128, HL, 256], BF16, name="pkv", tag="pst")
            for h in range(HL):
                nc.tensor.transpose(pkv[:, h, 0:128], raw[("v", h)][:, csl], id16)
                nc.tensor.transpose(pkv[:, h, 128:256], raw[("k", h)][:, csl], id16)
            vk = workp.tile([128, HL, 256], BF16, name="vk", tag="vk")
            nc.scalar.copy(vk, pkv)
            sqk = workp.tile([128, HL, 128], F32, name="sqk", tag="sqk")
            nc.gpsimd.tensor_mul(sqk, vk[:, :, 128:256], vk[:, :, 128:256])
            nc.vector.reduce_sum(nq[:, 0:4], sqk, axis=AX.X)
            return dict(cidx=cidx, csl=csl, vk=vk, nq=nq, bt=bt, raw=raw)

        def emit_SC(s_):
            """Scalar chain: rk, rq, beta', rkb (all [128(tok), 4(head)])."""
            nq, bt = s_["nq"], s_["bt"]
            nrm = workp.tile([128, 8], F32, name="nrm", tag="nrm")
            nc.scalar.activation(nrm, nq, AF.Sqrt)  # [k-norm | q-norm]
            nrme = workp.tile([128, 8], F32, name="nrme", tag="nrme")
            nc.vector.tensor_scalar_add(nrme, nrm, EPS)
            rkq = workp.tile([128, 8], F32, name="rkq", tag="rkq")
            nc.vector.reciprocal(rkq, nrme)
            t3 = workp.tile([128, 4], F32, name="t3", tag="t3")
            nc.vector.tensor_scalar_add(t3, bt, 1.0)
            rden = workp.tile([128, 4], F32, name="rden", tag="rden")
            nc.vector.reciprocal(rden, t3)
            bp = workp.tile([128, 4], F32, name="bp", tag="bp")
            nc.vector.tensor_mul(bp, bt, rden)
            rkb = workp.tile([128, 4], F32, name="rkb", tag="rkb")
            nc.vector.tensor_mul(rkb, rkq[:, 0:4], bp)
            s_["rk"] = rkq  # cols 0:4 rk, 4:8 rq
            s_["rkb"] = rkb

        def emit_A2(s_, opj=None):
            """Neumann chain: u = (I+N^4)(I+N^2)(I-N) (V' - K' S)."""
            csl, vk, raw = s_["csl"], s_["vk"], s_["raw"]
            rk, rkb = s_["rk"], s_["rkb"]

            kntm, lo, nt = {}, {}, {}
            x0 = workp.tile([128, HL, 256], BF16, name="x0", tag="x0", bufs=1)
            for h in range(HL):
                kt = workp.tile([128, 128], BF16, name="kntm", tag=f"kt{h}", bufs=1)
                nc.vector.tensor_scalar_mul(kt, vk[:, h, 128:256], rk[:, h:h + 1])
                kntm[h] = kt
                nc.vector.tensor_scalar_mul(x0[:, h, :], vk[:, h, :],
                                            rkb[:, h:h + 1])
            p3 = ps_ptx.tile([128, HL, 128], BF16, name="p3", tag="ptx")
            for h in range(HL):
                nc.tensor.transpose(p3[:, h, :], x0[:, h, 128:256], id16)
            kpt = workp.tile([128, HL, 128], BF16, name="kpt", tag="kpt", bufs=1)
            nc.scalar.copy(kpt, p3)
            pq = ps_mm.tile([128, HL, 128], F32, name="pkq", tag="mm")
            for h in range(HL):
                nc.tensor.matmul(pq[:, h, :], raw[("k", h)][:, csl],
                                 raw[("q", h)][:, csl])
            for h in range(HL):
                lo_t = workp.tile([128, 128], BF16, name="lo", tag=f"lo{h}", bufs=1)
                nc.vector.scalar_tensor_tensor(lo_t, pq[:, h, :], rk[:, h:h + 1],
                                               mk[:, 2, :], op0=ALU.mult, op1=ALU.mult)
                lo[h] = lo_t
            # R0 = V' - K' S
            pu = ps_mm.tile([128, HL, 128], F32, name="psu", tag="mm")
            for h in range(HL):
                nc.tensor.matmul(pu[:, h, :], kpt[:, h, :], s16[:, h, :])
            R = workp.tile([128, HL, 128], BF16, name="r0", tag="r0", bufs=2)
            nc.vector.tensor_sub(R, x0[:, :, 0:128], pu)
            pkk = ps_mm.tile([128, HL, 128], F32, name="pkk", tag="mm")
            for h in range(HL):
                nc.tensor.matmul(pkk[:, h, :], raw[("k", h)][:, csl], kpt[:, h, :])
            for h in range(HL):
                nt_t = workp.tile([128, 128], BF16, name="ntl", tag="ntl", bufs=4)
                nc.vector.scalar_tensor_tensor(nt_t, pkk[:, h, :], rk[:, h:h + 1],
                                               mk[:, 1, :], op0=ALU.mult, op1=ALU.mult)
                nt[h] = nt_t
            pnm = ps_ptx.tile([128, HL, 128], BF16, name="pnm", tag="ptx")
            for h in range(HL):
                nc.tensor.transpose(pnm[:, h, :], nt[h], id16)
            nm = workp.tile([128, HL, 128], BF16, name="nml", tag="nml", bufs=2)
            nc.scalar.copy(nm, pnm)
            # R1 = (I - N) R0
            pr = ps_mm.tile([128, HL, 128], F32, name="psx", tag="mm")
            for h in range(HL):
                nc.tensor.matmul(pr[:, h, :], nt[h], R[:, h, :])
            R1 = workp.tile([128, HL, 128], BF16, name="r1", tag="r1", bufs=2)
            nc.vector.tensor_sub(R1, R, pr)
            R = R1
            # t1 = (N^2)^T: the only power tile needed (z-chain form)
            pt1 = ps_mm.tile([128, HL, 128], F32, name="psc", tag="mm")
            for h in range(HL):
                nc.tensor.matmul(pt1[:, h, :], nm[:, h, :], nt[h])
            t1 = workp.tile([128, HL, 128], BF16, name="ct1", tag="ct1", bufs=2)
            nc.scalar.copy(t1, pt1)
            if opj:
                emit_outproj_oc(st[opj[0][0]], opj[0][1])  # filler while t1 drains
            # R2 = (I + N^2) R1
            pr = ps_mm.tile([128, HL, 128], F32, name="psx", tag="mm")
            for h in range(HL):
                nc.tensor.matmul(pr[:, h, :], t1[:, h, :], R[:, h, :])
            R2 = workp.tile([128, HL, 128], BF16, name="r2", tag="r2", bufs=2)
            nc.vector.tensor_add(R2, R, pr)
            R = R2
            # z = N^2 R2
            pz = ps_mm.tile([128, HL, 128], F32, name="psz", tag="mm")
            for h in range(HL):
                nc.tensor.matmul(pz[:, h, :], t1[:, h, :], R[:, h, :])
            z = workp.tile([128, HL, 128], BF16, name="zt", tag="zt", bufs=2)
            nc.scalar.copy(z, pz)
            if len(opj) > 1:
                emit_outproj_oc(st[opj[1][0]], opj[1][1])
            # u = R2 + N^2 z = (I + N^4) R2
            pr = ps_mm.tile([128, HL, 128], F32, name="psx", tag="mm")
            for h in range(HL):
                nc.tensor.matmul(pr[:, h, :], t1[:, h, :], z[:, h, :])
            u = workp.tile([128, HL, 128], BF16, name="u", tag="u", bufs=2)
            nc.vector.tensor_add(u, R, pr)
            s_["u"] = u
            s_["kntm"] = kntm
            s_["lo"] = lo

        def emit_B1(s_):
            """Outputs po, state update, RMS stats."""
            csl, raw, u, kntm, lo, rk = (
                s_["csl"], s_["raw"], s_["u"], s_["kntm"], s_["lo"], s_["rk"])
            po = ps_po.tile([128, HL, 128], F32, name="pso", tag="po")
            for h in range(HL):
                nc.tensor.matmul(po[:, h, :], lo[h], u[:, h, :],
                                 start=True, stop=False)
                nc.tensor.matmul(po[:, h, :], raw[("q", h)][:, csl], s16[:, h, :],
                                 start=False, stop=True)
            ob0 = workp.tile([128, HL, 128], BF16, name="ob0", tag="ob0", bufs=2)
            nc.scalar.copy(ob0, po)
            pd = ps_mm.tile([128, HL, 128], F32, name="psd", tag="mm")
            for h in range(HL):
                nc.tensor.matmul(pd[:, h, :], kntm[h], u[:, h, :])
            nc.vector.tensor_add(s32, s32, pd)
            msb = workp.tile([128, 4], F32, name="msb", tag="msb")
            sqo = workp.tile([128, HL, 128], F32, name="sqo", tag="sqo")
            nc.gpsimd.tensor_mul(sqo, ob0, ob0)
            nc.vector.reduce_sum(msb, sqo, axis=AX.X)
            nc.scalar.copy(s16, s32)
            # ro = rq / sqrt(mean(o^2 rq^2) + eps)
            ms1 = workp.tile([128, 4], F32, name="ms1", tag="ms1")
            nc.vector.tensor_mul(ms1, msb, rk[:, 4:8])
            nc.vector.tensor_mul(ms1, ms1, rk[:, 4:8])
            ms2 = workp.tile([128, 4], F32, name="ms2", tag="ms2")
            nc.scalar.activation(ms2, ms1, AF.Sqrt, scale=1.0 / HD, bias=epsb)
            rr = workp.tile([128, 4], F32, name="rr", tag="rr")
            nc.vector.reciprocal(rr, ms2)
            ro = workp.tile([128, 4], F32, name="ro", tag="ro")
            nc.vector.tensor_mul(ro, rr, rk[:, 4:8])
            s_["ob0"] = ob0
            s_["ro"] = ro

        def emit_B2(s_):
            ob0, ro = s_["ob0"], s_["ro"]
            onb = workp.tile([128, HL, 128], BF16, name="onb", tag="onb", bufs=2)
            for h in range(HL):
                nc.vector.tensor_scalar_mul(onb[:, h, :], ob0[:, h, :],
                                            ro[:, h:h + 1])
            ot = workp.tile([128, HL, 128], BF16, name="ot", tag="ot", bufs=2)
            for h in range(HL):
                nc.sync.dma_start_transpose(ot[:, h, :], onb[:, h, :])
            s_["ot"] = ot

        def emit_outproj_oc(s_, oc):
            cidx, ot = s_["cidx"], s_["ot"]
            tok = bass.ds(cidx * C, C)
            p = ps_op.tile([128, 512], F32, name="psop", tag="op")
            for h in range(HL):
                nc.tensor.matmul(p, ot[:, h, :],
                                 wo[:, h, oc * 512:(oc + 1) * 512],
                                 start=(h == 0), stop=(h == HL - 1))
            so = outp.tile([128, 512], F32, name="ost", tag="ost")
            nc.scalar.copy(so, p)
            for q in range(4):
                qs = bass.ds(oc * 512 + q * 128, 128)
                nc.sync.dma_start(out=out_d[tok, qs], in_=so[:, q * 128:(q + 1) * 128])

        # ---- schedule ----
        for p in range(3):
            emit_proj_piece(0, p)
        st[0] = emit_A1(0)
        emit_SC(st[0])
        for c in range(NCH):
            lt = c // 4
            if lt + 1 < NLT and c % 4 < 3:
                emit_proj_piece(lt + 1, c % 4)
            if c + 1 < NCH:
                st[c + 1] = emit_A1(c + 1)
                emit_SC(st[c + 1])
            emit_A2(st[c], opj=[(c - 1, 0), (c - 1, 1)] if c > 0 else [])
            emit_B1(st[c])
            emit_B2(st[c])
        emit_outproj_oc(st[NCH - 1], 0)
        emit_outproj_oc(st[NCH - 1], 1)
        for cc in range(NCH):
            st[cc] = None

    nc.compile()
    return nc


# ---------------- host side ----------------

def _bf(x):
    return np.ascontiguousarray(np.asarray(x, np.float32)).astype(ml_dtypes.bfloat16)


def host_prep(inputs):
    x = np.asarray(inputs["x"], np.float32)
    rms_vec = np.tile(np.asarray(inputs["rms_w"], np.float32), H)
    wo_eff = np.asarray(inputs["Wo"], np.float32) * rms_vec[None, :]

    masks = np.stack([
        np.tril(np.ones((128, 128), np.float32), -1),
        np.triu(np.ones((128, 128), np.float32), 1),
        np.triu(np.ones((128, 128), np.float32), 0),
    ]).astype(np.float32)
    ident = np.eye(128, dtype=np.float32)
    oneh = np.zeros((4, 128, 4), np.float32)
    for h in range(4):
        oneh[h, :, h] = 1.0

    for nm in ("bq", "bk", "bv", "bbeta", "bo", "convb_q", "convb_k", "convb_v"):
        assert np.all(np.asarray(inputs[nm]) == 0.0), f"nonzero bias {nm} unsupported"

    in_maps = []
    for c in range(8):
        b, hh = c // 2, c % 2
        rows = slice(hh * 512, (hh + 1) * 512)
        cds = []
        for s in ("k", "q", "v"):
            cw = np.asarray(inputs[f"conv_{s}"], np.float32)[rows]
            for h in range(HL):
                cds.append(np.stack([np.diag(cw[h * 128:(h + 1) * 128, j])
                                     for j in range(CONV)]))
        m = {
            "xt": _bf(x[b].T.reshape(KS, 128, L)),
            "wq": _bf(np.asarray(inputs["Wq"], np.float32)[rows].T.reshape(KS, 128, 512)),
            "wk": _bf(np.asarray(inputs["Wk"], np.float32)[rows].T.reshape(KS, 128, 512)),
            "wv": _bf(np.asarray(inputs["Wv"], np.float32)[rows].T.reshape(KS, 128, 512)),
            "wb": _bf(np.asarray(inputs["Wbeta"], np.float32)[hh * 4:(hh + 1) * 4].T.reshape(KS, 128, 4)),
            "wo": _bf(wo_eff[:, rows].T.reshape(4, 128, 1024)),
            "cd": np.stack(cds).astype(ml_dtypes.bfloat16),
            "mk": masks,
            "oh": _bf(oneh),
            "id16": _bf(ident),
            "id32": ident,
        }
        in_maps.append(m)
    return in_maps


def host_combine(results, inputs):
    bo = np.asarray(inputs["bo"], np.float32)
    out = np.zeros((B, L, D), np.float32)
    for b in range(B):
        out[b] = results[2 * b]["out"] + results[2 * b + 1]["out"] + bo
    return out


# ---------------- entry point ----------------

_NC_CACHE = []


def kernel(**inputs):
    """Full-input DeltaNet layer distributed over 8 NeuronCores.

    Shards batch (4) x head-group (2) across cores, runs the Bass kernel via
    run_bass_kernel_spmd, and reduces the per-pair partial out-projections on
    the host (the pair all-reduce) before returning [4, 2048, 1024] fp32.
    """
    from concourse.bass_utils import run_bass_kernel_spmd

    if not _NC_CACHE:
        _NC_CACHE.append(build_nc())
    nc = _NC_CACHE[0]
    in_maps = host_prep(inputs)
    br = run_bass_kernel_spmd(nc, in_maps, list(range(8)))
    return host_combine(br.results, inputs)


# revision 45
# speedup vs baseline: 1.0731x; 1.0731x over previous
"""Chunked DeltaNet layer on 8 TRN2 NeuronCores.

Sharding: core c -> batch b = c//2, head-group hh = c%2 (heads hh*4..hh*4+3).
Each core: q/k/v projections + causal conv + normalization for its 4 heads,
chunked (WY-form) delta rule over L=2048 in 16 chunks of 128, RMS norm,
partial out-projection [2048, 1024] (contraction over its 512 local o-dims).
Host sums the two partials per batch and adds bo.

Chunk math (per head, state S = A^T in [d_k, d_v]):
  N    = tril(K' K^T, -1)          K' = beta'*K_n (row scale), K_n = K/||K||
  Minv ~= (I+N^4)(I+N^2)(I-N)      [error ~N^8, ~5e-4 on this data]
  U    = Minv (V' - K' S)          (R-form: chain runs on one 128-wide tile)
  O    = diag(r_q) [tril(Q_raw K_n^T, 0) U + Q_raw S]
  S   += K_n^T U

Engine split: PE matmuls/transposes; DVE normalization scales, Neumann adds,
row-sum reductions (tensor_tensor_reduce); ACT psum drains + sigmoid/sqrt;
GpSimd bulk psum->SBUF copies (vkt/s16/ot).

Emission is software-pipelined per chunk c:
  proj-piece(lt+1) | A1(c+1) transposes+grams | SC(c+1) scalars |
  A2(c) Neumann chain | B1(c) outputs+state | outproj(c-1) | B2(c) RMS drain
so the PE FIFO always holds ready work while ACT/DVE chains resolve.
"""

import contextlib
import os

import ml_dtypes
import numpy as np

import concourse.bass as bass
import concourse.mybir as mybir
import concourse.tile as tile
from concourse import bacc

F32 = mybir.dt.float32
BF16 = mybir.dt.bfloat16
AF = mybir.ActivationFunctionType
ALU = mybir.AluOpType
AX = mybir.AxisListType

B, L, D, H, HD, CONV = 4, 2048, 1024, 8, 128, 4
ETA, EPS = 1.0, 1e-6
C = 128
NCH = L // C
NLT = 4
LT = 512
HL = 4
KS = D // 128
SIG = ("k", "q", "v")


def build_nc():
    nc = bacc.Bacc("TRN2", target_bir_lowering=False, debug=False)

    xt_d = nc.dram_tensor("xt", [KS, 128, L], BF16, kind="ExternalInput").ap()
    wq_d = nc.dram_tensor("wq", [KS, 128, 512], BF16, kind="ExternalInput").ap()
    wk_d = nc.dram_tensor("wk", [KS, 128, 512], BF16, kind="ExternalInput").ap()
    wv_d = nc.dram_tensor("wv", [KS, 128, 512], BF16, kind="ExternalInput").ap()
    wb_d = nc.dram_tensor("wb", [KS, 128, 4], BF16, kind="ExternalInput").ap()
    wo_d = nc.dram_tensor("wo", [4, 128, 1024], BF16, kind="ExternalInput").ap()
    cd_d = nc.dram_tensor("cd", [12, CONV, 128, 128], BF16, kind="ExternalInput").ap()
    mk_d = nc.dram_tensor("mk", [3, 128, 128], F32, kind="ExternalInput").ap()
    oh_d = nc.dram_tensor("oh", [4, 128, 4], BF16, kind="ExternalInput").ap()
    id16_d = nc.dram_tensor("id16", [128, 128], BF16, kind="ExternalInput").ap()
    id32_d = nc.dram_tensor("id32", [128, 128], F32, kind="ExternalInput").ap()
    out_d = nc.dram_tensor("out", [L, D], F32, kind="ExternalOutput").ap()

    with tile.TileContext(nc) as tc, contextlib.ExitStack() as ctx:
        consts = ctx.enter_context(tc.tile_pool(name="consts", bufs=1))
        persist = ctx.enter_context(tc.tile_pool(name="persist", bufs=1))
        projp = ctx.enter_context(tc.tile_pool(name="projp", bufs=2))
        rawp = ctx.enter_context(tc.tile_pool(name="rawp", bufs=2))
        sqp = ctx.enter_context(tc.tile_pool(name="sqp", bufs=2))
        workp = ctx.enter_context(tc.tile_pool(name="workp", bufs=2))
        outp = ctx.enter_context(tc.tile_pool(name="outp", bufs=2))
        # PSUM: 8 banks of 2KB/partition, each pool buf = 1 bank.
        # big 2 + pst 1 + kq 1 + mm 2 + po 1 + ptx 1 = 8 banks.
        # Per-head [128,128] outputs pack into [128,4,128] bank tiles.
        ps_big = ctx.enter_context(tc.tile_pool(name="ps_big", bufs=2, space="PSUM"))
        ps_pst = ctx.enter_context(tc.tile_pool(name="ps_pst", bufs=1, space="PSUM"))
        ps_op = ctx.enter_context(tc.tile_pool(name="ps_op", bufs=1, space="PSUM"))
        ps_mm = ctx.enter_context(tc.tile_pool(name="ps_mm", bufs=2, space="PSUM"))
        ps_po = ctx.enter_context(tc.tile_pool(name="ps_po", bufs=1, space="PSUM"))
        ps_ptx = ctx.enter_context(tc.tile_pool(name="ps_ptx", bufs=1, space="PSUM"))

        # ---- constants, DMA-ordered by first use ----
        xt = consts.tile([128, KS, L], BF16)
        ws = {}
        for name in ("k", "q", "v"):
            ws[name] = consts.tile([128, KS, 512], BF16, name=f"w{name}")
        cd = consts.tile([128, 12, CONV, 128], BF16)
        mk = consts.tile([128, 3, 128], F32)
        oh = consts.tile([128, 4, 4], BF16)
        id16 = consts.tile([128, 128], BF16)
        id32 = consts.tile([128, 128], F32)
        wb = consts.tile([128, KS, 4], BF16)
        wo = consts.tile([128, 4, 1024], BF16)

        wd = {"k": wk_d, "q": wq_d, "v": wv_d}
        for i in range(KS):
            for sp in range(2):
                ss = bass.ds(sp * (L // 2), L // 2)
                nc.sync.dma_start(out=xt[:, i, ss], in_=xt_d[i][:, ss])
            nc.sync.dma_start(out=ws["k"][:, i, :], in_=wk_d[i])
        for n_ in range(4):
            for j_ in range(CONV):
                nc.sync.dma_start(out=cd[:, n_, j_, :], in_=cd_d[n_, j_])
        nc.sync.dma_start(out=id16, in_=id16_d)
        nc.sync.dma_start(out=id32, in_=id32_d)
        for n_ in range(3):
            nc.sync.dma_start(out=mk[:, n_, :], in_=mk_d[n_])
        for i in range(KS):
            nc.sync.dma_start(out=ws["q"][:, i, :], in_=wq_d[i])
        for n_ in range(4, 8):
            for j_ in range(CONV):
                nc.sync.dma_start(out=cd[:, n_, j_, :], in_=cd_d[n_, j_])
        for i in range(KS):
            nc.sync.dma_start(out=ws["v"][:, i, :], in_=wv_d[i])
        for n_ in range(8, 12):
            for j_ in range(CONV):
                nc.sync.dma_start(out=cd[:, n_, j_, :], in_=cd_d[n_, j_])
        for n_ in range(4):
            nc.sync.dma_start(out=oh[:, n_, :], in_=oh_d[n_])
        for i in range(KS):
            nc.sync.dma_start(out=wb[:, i, :], in_=wb_d[i])
        for i in range(4):
            nc.sync.dma_start(out=wo[:, i, :], in_=wo_d[i])

        # ---- persistent ----
        beta_cm = persist.tile([4, L], F32)  # sigmoid(beta)
        qn2_cm = persist.tile([4, L], F32)   # ||q_raw||^2
        s32 = persist.tile([128, HL, 128], F32)
        s16 = persist.tile([128, HL, 128], BF16)
        halo = persist.tile([128, 12, 4], BF16)  # conv halo between lt blocks
        nc.vector.memset(s32, 0.0)
        nc.vector.memset(s16, 0.0)
        epsb = persist.tile([128, 1], F32)
        nc.vector.memset(epsb, EPS)

        raws = [{} for _ in range(NLT)]
        sqs = [{} for _ in range(NLT)]
        st = [None] * NCH  # per-chunk state dicts

        # one sig per piece so ACT Sigmoid ops cluster (fewer table loads)
        PIECES = [[("k", h) for h in range(HL)], [("q", h) for h in range(HL)],
                  [("v", h) for h in range(HL)], []]

        def emit_proj_piece(lt, piece):
            """Projection+conv for one sig (4 heads); piece 2 adds beta/qn2."""
            tsl = bass.ds(lt * LT, LT)
            raw = raws[lt]
            items = PIECES[piece]
            pts = []
            for s, h in items:
                n = SIG.index(s) * HL + h
                ps = ps_big.tile([128, LT], F32, name="psproj", tag="big")
                for i in range(KS):
                    nc.tensor.matmul(
                        ps, ws[s][:, i, h * 128:(h + 1) * 128], xt[:, i, tsl],
                        start=(i == 0), stop=(i == KS - 1))
                pt = projp.tile([128, LT + 4], BF16, name="pt", tag=f"pj{s}{h}",
                                bufs=1)
                if lt == 0:
                    nc.scalar.memzero(pt[:, 0:4])
                else:
                    nc.scalar.copy(pt[:, 0:3], halo[:, n, 0:3])
                nc.scalar.copy(pt[:, 3:LT + 3], ps)
                if lt + 1 < NLT:
                    nc.scalar.copy(halo[:, n, 0:3], pt[:, LT:LT + 3])
                pts.append(pt)
            for (s, h), pt in zip(items, pts):
                n = SIG.index(s) * HL + h
                pc = ps_big.tile([128, LT], F32, name="psconv", tag="big")
                for j in range(CONV):
                    nc.tensor.matmul(pc, cd[:, n, j, :], pt[:, j:LT + j],
                                     start=(j == 0), stop=(j == CONV - 1))
                r = rawp.tile([128, LT], BF16, name="raw", tag=f"rw{s}{h}")
                if s == "v":
                    vsg = sqp.tile([128, LT], BF16, name="vsig", tag=f"vg{h}",
                                   bufs=1)
                    nc.scalar.activation(vsg, pc, AF.Sigmoid)
                    nc.vector.tensor_mul(r, pc, vsg)  # silu(v)
                else:
                    nc.scalar.copy(r, pc)
                    if s == "q":
                        sq = sqp.tile([128, LT], BF16, name="sq", tag=f"sq{h}",
                                      bufs=1)
                        nc.vector.tensor_mul(sq, r, r)
                        sqs[lt][h] = sq
                raw[(s, h)] = r
            if piece == 2:
                psq = ps_big.tile([128, LT], F32, name="psq", tag="big")
                for h in range(HL):
                    nc.tensor.matmul(psq[0:4, :], oh[:, h, :], sqs[lt][h],
                                     start=(h == 0), stop=(h == HL - 1))
                nc.vector.tensor_copy(qn2_cm[:, tsl], psq[0:4, :])
                psb = ps_big.tile([128, LT], F32, name="psbeta", tag="big")
                for i in range(KS):
                    nc.tensor.matmul(psb[0:4, :], wb[:, i, :], xt[:, i, tsl],
                                     start=(i == 0), stop=(i == KS - 1))
                nc.scalar.activation(beta_cm[:, tsl], psb[0:4, :], AF.Sigmoid)

        def emit_A1(cidx):
            """Transposes + norm stats for chunk cidx (independent of scalars)."""
            lt = cidx // 4
            raw = raws[lt]
            csl = bass.ds((cidx % 4) * C, C)
            gsl = bass.ds(cidx * C, C)

            psbt = ps_mm.tile([128, 8], F32, name="psbt", tag="mm")
            nc.tensor.transpose(psbt[:, 0:4], beta_cm[:, gsl], id32[0:4, 0:4])
            nc.tensor.transpose(psbt[:, 4:8], qn2_cm[:, gsl], id32[0:4, 0:4])

            nq = workp.tile([128, 8], F32, name="nq", tag="nq")
            nc.vector.tensor_copy(nq[:, 4:8], psbt[:, 4:8])
            bt = workp.tile([128, 4], F32, name="bt", tag="bt")
            nc.vector.tensor_copy(bt, psbt[:, 0:4])

            pkv = ps_pst.tile([128, HL, 256], BF16, name="pkv", tag="pst")
            for h in range(HL):
                nc.tensor.transpose(pkv[:, h, 0:128], raw[("v", h)][:, csl], id16)
                nc.tensor.transpose(pkv[:, h, 128:256], raw[("k", h)][:, csl], id16)
            vk = workp.tile([128, HL, 256], BF16, name="vk", tag="vk")
            nc.scalar.copy(vk, pkv)
            sqk = workp.tile([128, HL, 128], F32, name="sqk", tag="sqk")
            nc.gpsimd.tensor_mul(sqk, vk[:, :, 128:256], vk[:, :, 128:256])
            nc.vector.reduce_sum(nq[:, 0:4], sqk, axis=AX.X)
            return dict(cidx=cidx, csl=csl, vk=vk, nq=nq, bt=bt, raw=raw)

        def emit_SC(s_):
            """Scalar chain: rk, rq, beta', rkb (all [128(tok), 4(head)])."""
            nq, bt = s_["nq"], s_["bt"]
            nrm = workp.tile([128, 8], F32, name="nrm", tag="nrm")
            nc.scalar.activation(nrm, nq, AF.Sqrt)  # [k-norm | q-norm]
            nrme = workp.tile([128, 8], F32, name="nrme", tag="nrme")
            nc.vector.tensor_scalar_add(nrme, nrm, EPS)
            rkq = workp.tile([128, 8], F32, name="rkq", tag="rkq")
            nc.vector.reciprocal(rkq, nrme)
            t3 = workp.tile([128, 4], F32, name="t3", tag="t3")
            nc.vector.tensor_scalar_add(t3, bt, 1.0)
            rden = workp.tile([128, 4], F32, name="rden", tag="rden")
            nc.vector.reciprocal(rden, t3)
            bp = workp.tile([128, 4], F32, name="bp", tag="bp")
            nc.vector.tensor_mul(bp, bt, rden)
            rkb = workp.tile([128, 4], F32, name="rkb", tag="rkb")
            nc.vector.tensor_mul(rkb, rkq[:, 0:4], bp)
            s_["rk"] = rkq  # cols 0:4 rk, 4:8 rq
            s_["rkb"] = rkb

        def emit_A2(s_, opj=None):
            """Neumann chain: u = (I+N^4)(I+N^2)(I-N) (V' - K' S)."""
            csl, vk, raw = s_["csl"], s_["vk"], s_["raw"]
            rk, rkb = s_["rk"], s_["rkb"]

            kntm, lo, nt = {}, {}, {}
            x0 = workp.tile([128, HL, 256], BF16, name="x0", tag="x0", bufs=1)
            for h in range(HL):
                kt = workp.tile([128, 128], BF16, name="kntm", tag=f"kt{h}", bufs=1)
                nc.vector.tensor_scalar_mul(kt, vk[:, h, 128:256], rk[:, h:h + 1])
                kntm[h] = kt
                nc.vector.tensor_scalar_mul(x0[:, h, :], vk[:, h, :],
                                            rkb[:, h:h + 1])
            p3 = ps_ptx.tile([128, HL, 128], BF16, name="p3", tag="ptx")
            for h in range(HL):
                nc.tensor.transpose(p3[:, h, :], x0[:, h, 128:256], id16)
            kpt = workp.tile([128, HL, 128], BF16, name="kpt", tag="kpt", bufs=1)
            nc.scalar.copy(kpt, p3)
            pq = ps_mm.tile([128, HL, 128], F32, name="pkq", tag="mm")
            for h in range(HL):
                nc.tensor.matmul(pq[:, h, :], raw[("k", h)][:, csl],
                                 raw[("q", h)][:, csl])
            for h in range(HL):
                lo_t = workp.tile([128, 128], BF16, name="lo", tag=f"lo{h}", bufs=1)
                nc.vector.scalar_tensor_tensor(lo_t, pq[:, h, :], rk[:, h:h + 1],
                                               mk[:, 2, :], op0=ALU.mult, op1=ALU.mult)
                lo[h] = lo_t
            # R0 = V' - K' S
            pu = ps_mm.tile([128, HL, 128], F32, name="psu", tag="mm")
            for h in range(HL):
                nc.tensor.matmul(pu[:, h, :], kpt[:, h, :], s16[:, h, :])
            R = workp.tile([128, HL, 128], BF16, name="r0", tag="r0", bufs=2)
            nc.vector.tensor_sub(R, x0[:, :, 0:128], pu)
            pkk = ps_mm.tile([128, HL, 128], F32, name="pkk", tag="mm")
            for h in range(HL):
                nc.tensor.matmul(pkk[:, h, :], raw[("k", h)][:, csl], kpt[:, h, :])
            for h in range(HL):
                nt_t = workp.tile([128, 128], BF16, name="ntl", tag="ntl", bufs=4)
                nc.vector.scalar_tensor_tensor(nt_t, pkk[:, h, :], rk[:, h:h + 1],
                                               mk[:, 1, :], op0=ALU.mult, op1=ALU.mult)
                nt[h] = nt_t
            pnm = ps_ptx.tile([128, HL, 128], BF16, name="pnm", tag="ptx")
            for h in range(HL):
                nc.tensor.transpose(pnm[:, h, :], nt[h], id16)
            nm = workp.tile([128, HL, 128], BF16, name="nml", tag="nml", bufs=2)
            nc.scalar.copy(nm, pnm)
            # R1 = (I - N) R0
            pr = ps_mm.tile([128, HL, 128], F32, name="psx", tag="mm")
            for h in range(HL):
                nc.tensor.matmul(pr[:, h, :], nt[h], R[:, h, :])
            R1 = workp.tile([128, HL, 128], BF16, name="r1", tag="r1", bufs=2)
            nc.vector.tensor_sub(R1, R, pr)
            R = R1
            # t1 = (N^2)^T: the only power tile needed (z-chain form)
            pt1 = ps_mm.tile([128, HL, 128], F32, name="psc", tag="mm")
            for h in range(HL):
                nc.tensor.matmul(pt1[:, h, :], nm[:, h, :], nt[h])
            t1 = workp.tile([128, HL, 128], BF16, name="ct1", tag="ct1", bufs=2)
            nc.scalar.copy(t1, pt1)
            if opj:
                emit_outproj_oc(st[opj[0][0]], opj[0][1])  # filler while t1 drains
            # R2 = (I + N^2) R1
            pr = ps_mm.tile([128, HL, 128], F32, name="psx", tag="mm")
            for h in range(HL):
                nc.tensor.matmul(pr[:, h, :], t1[:, h, :], R[:, h, :])
            R2 = workp.tile([128, HL, 128], BF16, name="r2", tag="r2", bufs=2)
            nc.vector.tensor_add(R2, R, pr)
            R = R2
            # z = N^2 R2
            pz = ps_mm.tile([128, HL, 128], F32, name="psz", tag="mm")
            for h in range(HL):
                nc.tensor.matmul(pz[:, h, :], t1[:, h, :], R[:, h, :])
            z = workp.tile([128, HL, 128], BF16, name="zt", tag="zt", bufs=2)
            nc.scalar.copy(z, pz)
            if len(opj) > 1:
                emit_outproj_oc(st[opj[1][0]], opj[1][1])
            # u = R2 + N^2 z = (I + N^4) R2
            pr = ps_mm.tile([128, HL, 128], F32, name="psx", tag="mm")
            for h in range(HL):
                nc.tensor.matmul(pr[:, h, :], t1[:, h, :], z[:, h, :])
            u = workp.tile([128, HL, 128], BF16, name="u", tag="u", bufs=2)
            nc.vector.tensor_add(u, R, pr)
            s_["u"] = u
            s_["kntm"] = kntm
            s_["lo"] = lo

        def emit_B1(s_):
            """Outputs po, state update, RMS stats."""
            csl, raw, u, kntm, lo, rk = (
                s_["csl"], s_["raw"], s_["u"], s_["kntm"], s_["lo"], s_["rk"])
            po = ps_po.tile([128, HL, 128], F32, name="pso", tag="po")
            for h in range(HL):
                nc.tensor.matmul(po[:, h, :], lo[h], u[:, h, :],
                                 start=True, stop=False)
                nc.tensor.matmul(po[:, h, :], raw[("q", h)][:, csl], s16[:, h, :],
                                 start=False, stop=True)
            ob0 = workp.tile([128, HL, 128], BF16, name="ob0", tag="ob0", bufs=2)
            nc.scalar.copy(ob0, po)
            pd = ps_mm.tile([128, HL, 128], F32, name="psd", tag="mm")
            for h in range(HL):
                nc.tensor.matmul(pd[:, h, :], kntm[h], u[:, h, :])
            nc.vector.tensor_add(s32, s32, pd)
            msb = workp.tile([128, 4], F32, name="msb", tag="msb")
            sqo = workp.tile([128, HL, 128], F32, name="sqo", tag="sqo")
            nc.gpsimd.tensor_mul(sqo, ob0, ob0)
            nc.vector.reduce_sum(msb, sqo, axis=AX.X)
            nc.scalar.copy(s16, s32)
            # ro = rq / sqrt(mean(o^2 rq^2) + eps)
            ms1 = workp.tile([128, 4], F32, name="ms1", tag="ms1")
            nc.vector.tensor_mul(ms1, msb, rk[:, 4:8])
            nc.vector.tensor_mul(ms1, ms1, rk[:, 4:8])
            ms2 = workp.tile([128, 4], F32, name="ms2", tag="ms2")
            nc.scalar.activation(ms2, ms1, AF.Sqrt, scale=1.0 / HD, bias=epsb)
            rr = workp.tile([128, 4], F32, name="rr", tag="rr")
            nc.vector.reciprocal(rr, ms2)
            ro = workp.tile([128, 4], F32, name="ro", tag="ro")
            nc.vector.tensor_mul(ro, rr, rk[:, 4:8])
            s_["ob0"] = ob0
            s_["ro"] = ro

        def emit_B2(s_):
            ob0, ro = s_["ob0"], s_["ro"]
            onb = workp.tile([128, HL, 128], BF16, name="onb", tag="onb", bufs=2)
            for h in range(HL):
                nc.vector.tensor_scalar_mul(onb[:, h, :], ob0[:, h, :],
                                            ro[:, h:h + 1])
            pot = ps_ptx.tile([128, HL, 128], BF16, name="psot", tag="ptx")
            for h in range(HL):
                nc.tensor.transpose(pot[:, h, :], onb[:, h, :], id16)
            ot = workp.tile([128, HL, 128], BF16, name="ot", tag="ot",
                             bufs=8)
            nc.scalar.copy(ot, pot)
            s_["ot"] = ot

        def emit_outproj_oc(s_, oc):
            cidx, ot = s_["cidx"], s_["ot"]
            tok = bass.ds(cidx * C, C)
            p = ps_op.tile([128, 512], F32, name="psop", tag="op")
            for h in range(HL):
                nc.tensor.matmul(p, ot[:, h, :],
                                 wo[:, h, oc * 512:(oc + 1) * 512],
                                 start=(h == 0), stop=(h == HL - 1))
            so = outp.tile([128, 512], F32, name="ost", tag="ost")
            nc.scalar.copy(so, p)
            for q in range(4):
                qs = bass.ds(oc * 512 + q * 128, 128)
                nc.sync.dma_start(out=out_d[tok, qs], in_=so[:, q * 128:(q + 1) * 128])

        # ---- schedule ----
        for p in range(3):
            emit_proj_piece(0, p)
        st[0] = emit_A1(0)
        emit_SC(st[0])
        for c in range(NCH):
            lt = c // 4
            if lt + 1 < NLT and c % 4 < 3:
                emit_proj_piece(lt + 1, c % 4)
            if c + 1 < NCH:
                st[c + 1] = emit_A1(c + 1)
                emit_SC(st[c + 1])
            emit_A2(st[c], opj=[(c - 1, 0), (c - 1, 1)] if c > 0 else [])
            emit_B1(st[c])
            emit_B2(st[c])
        emit_outproj_oc(st[NCH - 1], 0)
        emit_outproj_oc(st[NCH - 1], 1)
        for cc in range(NCH):
            st[cc] = None

    nc.compile()
    return nc


# ---------------- host side ----------------

def _bf(x):
    return np.ascontiguousarray(np.asarray(x, np.float32)).astype(ml_dtypes.bfloat16)


def host_prep(inputs):
    x = np.asarray(inputs["x"], np.float32)
    rms_vec = np.tile(np.asarray(inputs["rms_w"], np.float32), H)
    wo_eff = np.asarray(inputs["Wo"], np.float32) * rms_vec[None, :]

    masks = np.stack([
        np.tril(np.ones((128, 128), np.float32), -1),
        np.triu(np.ones((128, 128), np.float32), 1),
        np.triu(np.ones((128, 128), np.float32), 0),
    ]).astype(np.float32)
    ident = np.eye(128, dtype=np.float32)
    oneh = np.zeros((4, 128, 4), np.float32)
    for h in range(4):
        oneh[h, :, h] = 1.0

    for nm in ("bq", "bk", "bv", "bbeta", "bo", "convb_q", "convb_k", "convb_v"):
        assert np.all(np.asarray(inputs[nm]) == 0.0), f"nonzero bias {nm} unsupported"

    in_maps = []
    for c in range(8):
        b, hh = c // 2, c % 2
        rows = slice(hh * 512, (hh + 1) * 512)
        cds = []
        for s in ("k", "q", "v"):
            cw = np.asarray(inputs[f"conv_{s}"], np.float32)[rows]
            for h in range(HL):
                cds.append(np.stack([np.diag(cw[h * 128:(h + 1) * 128, j])
                                     for j in range(CONV)]))
        m = {
            "xt": _bf(x[b].T.reshape(KS, 128, L)),
            "wq": _bf(np.asarray(inputs["Wq"], np.float32)[rows].T.reshape(KS, 128, 512)),
            "wk": _bf(np.asarray(inputs["Wk"], np.float32)[rows].T.reshape(KS, 128, 512)),
            "wv": _bf(np.asarray(inputs["Wv"], np.float32)[rows].T.reshape(KS, 128, 512)),
            "wb": _bf(np.asarray(inputs["Wbeta"], np.float32)[hh * 4:(hh + 1) * 4].T.reshape(KS, 128, 4)),
            "wo": _bf(wo_eff[:, rows].T.reshape(4, 128, 1024)),
            "cd": np.stack(cds).astype(ml_dtypes.bfloat16),
            "mk": masks,
            "oh": _bf(oneh),
            "id16": _bf(ident),
            "id32": ident,
        }
        in_maps.append(m)
    return in_maps


def host_combine(results, inputs):
    bo = np.asarray(inputs["bo"], np.float32)
    out = np.zeros((B, L, D), np.float32)
    for b in range(B):
        out[b] = results[2 * b]["out"] + results[2 * b + 1]["out"] + bo
    return out


# ---------------- entry point ----------------

_NC_CACHE = []


def kernel(**inputs):
    """Full-input DeltaNet layer distributed over 8 NeuronCores.

    Shards batch (4) x head-group (2) across cores, runs the Bass kernel via
    run_bass_kernel_spmd, and reduces the per-pair partial out-projections on
    the host (the pair all-reduce) before returning [4, 2048, 1024] fp32.
    """
    from concourse.bass_utils import run_bass_kernel_spmd

    if not _NC_CACHE:
        _NC_CACHE.append(build_nc())
    nc = _NC_CACHE[0]
    in_maps = host_prep(inputs)
    br = run_bass_kernel_spmd(nc, in_maps, list(range(8)))
    return host_combine(br.results, inputs)
